# revision 1
# baseline (speedup 1.0000x reference)
"""Dynamic-conv (CondConv-style) kernel for Trainium2, 8 NeuronCores.

Problem: for each sample b:
    se     = global-avg-pool(x[b])                     (256,)
    gates  = sigmoid(se @ route_w.T + route_b)         (8,)
    w_dyn  = (gates @ weight.T).reshape(256,256,3,3)   per-sample 3x3 conv kernel
    out[b] = conv2d(x[b], w_dyn, pad=1) + bias         (256,28,28)

Sharding: data-parallel over batch, 4 samples per core; the expert weight
bank and routing weights are replicated.

Per-core plan:
  - x: host-packed partition-major [128, s, t, 28, 28] bf16 so each sample
    is one DMA with 128 large contiguous descriptors; fused DVE pass gives
    the pooled `se` (no zero-pad stage: conv uses edge-clipped matmuls).
  - gates: tiny fp32 matmuls against a 16x-replicated routing matrix produce
    a 128x64 block of logits; sigmoid (ACT) + block-diagonal mask (DVE)
    yield a block-diagonal stationary G[(q,e),(s,q)] = gate[s,e] * (q==q').
  - bank: 6 DMAs of [128, 16, 384] (12.3KB/partition contiguous), issued on
    the sync queue AFTER x so x wins the FIFO race (gates are the critical
    path). SDMA engines round-robin descriptors with no priority, so the
    sync-ring FIFO is the only ordering lever.
  - kernel synthesis on the TensorEngine: a rhs tile's partition p=(q,e)
    carries 16 ci-chunks of all 8 experts; one matmul against G computes
    sum_e gate[s,e]*W_e for 16 chunks x 4 samples at once.
  - PSUM (64,384) tiles drain (cast bf16) alternately on ACT/DVE into a
    (128, 8, 1152) SBUF stage; flat SBUF->SBUF DMAs per (sample, ci_tile,
    u) re-gather w_dyn with ci on partitions (contiguous 2.3KB/partition),
    all on the SCALAR HWDGE ring so they never queue behind the bank
    stream on the sync ring.
  - conv: 9 shifted matmuls (bf16) accumulating over (ci_tile, kh, kw) in
    PSUM, with k=4 (center) first so partial edge-clipped views accumulate
    into initialized PSUM; ACT adds bias on the PSUM->SBUF drain (bf16);
    DMA out bf16, host casts to fp32.
  - split into halves by output-channel block so conv(half A) overlaps the
    synthesis of half B.
"""

import os
from contextlib import ExitStack

import ml_dtypes
import numpy as np

import concourse.bacc as bacc
import concourse.bass as bass
import concourse.mybir as mybir
import concourse.tile as tile
from concourse.bass_utils import run_bass_kernel_spmd

FP32 = mybir.dt.float32
BF16 = mybir.dt.bfloat16
BF16_NP = ml_dtypes.bfloat16

N_CORES = 8
B, C_IN, H, W = 32, 256, 28, 28
NUM, C_OUT, K = 8, 256, 3
BS = B // N_CORES          # samples per core = 4
NQ = 16                    # ci chunks in the synthesis contraction
F = 2304                   # f = co_t*1152 + khkw*128 + co_lo
NWIN = 384                 # synthesis matmul free size


def build_nc() -> bacc.Bacc:
    nc = bacc.Bacc("TRN2", target_bir_lowering=False, debug=False,
                   num_devices=N_CORES)

    # x host-packed partition-major: [p, s, t, h, w], channel = t*128 + perm(p)
    x_d = nc.dram_tensor("x", [128, BS, 2, H, W], BF16, kind="ExternalInput")
    # bank[p=(q,e), w, cl, n] = W[e, ci=q*16+cl, f=w*384+n]; per-partition
    # contiguous so one DMA per w moves 12.3KB/partition runs.
    bank_d = nc.dram_tensor("bank", [128, 6, NQ, NWIN], BF16, kind="ExternalInput")
    cst_d = nc.dram_tensor("cst", [128, 323], FP32, kind="ExternalInput")
    out_d = nc.dram_tensor("out", [BS, C_OUT, H, W], BF16, kind="ExternalOutput")

    with tile.TileContext(nc) as tc, ExitStack() as ctx:
        singles = ctx.enter_context(tc.tile_pool(name="singles", bufs=1))
        bankp = ctx.enter_context(tc.tile_pool(name="bankp", bufs=6))
        stagep = ctx.enter_context(tc.tile_pool(name="stagep", bufs=2))
        wdynp = ctx.enter_context(tc.tile_pool(name="wdynp", bufs=1))
        outp = ctx.enter_context(tc.tile_pool(name="outp", bufs=2))
        psS = ctx.enter_context(tc.tile_pool(name="psS", bufs=4, space="PSUM"))
        psC = ctx.enter_context(tc.tile_pool(name="psC", bufs=4, space="PSUM"))

        # ---- replicated constants: one blob DMA (fewer tiny descriptors
        # ahead of x on the sync FIFO)
        cst = singles.tile([128, 323], FP32)
        nc.sync.dma_start(out=cst, in_=cst_d[:])
        rbx = cst[:, 256:257]
        biasT = cst[:, 257:259]
        mask = cst[:, 259:323]
        ones16 = singles.tile([128, NQ], FP32)
        nc.vector.memset(ones16, 1.0)
        warm = singles.tile([128, 1], FP32)
        nc.scalar.activation(out=warm, in_=ones16[:, 0:1],
                             func=mybir.ActivationFunctionType.Sigmoid)
        nc.scalar.activation(out=warm, in_=warm,
                             func=mybir.ActivationFunctionType.Identity,
                             bias=warm, scale=1.0)
        nc.scalar.activation(out=warm, in_=warm,
                             func=mybir.ActivationFunctionType.Copy)

        # ---- x in (4 big DMAs, one per sample), then bank (6 DMAs) on the
        # same sync FIFO so x lands first.
        xall = singles.tile([128, BS, 2, H, W], BF16)
        for s in range(BS):
            nc.sync.dma_start(out=xall[:, s], in_=x_d[:, s])
        bkt = {}
        for w in range(3):
            bk = bankp.tile([128, NQ, NWIN], BF16, tag="bk", name=f"bk{w}")
            nc.sync.dma_start(out=bk, in_=bank_d[:, w])
            bkt[w] = bk

        # ---- pooled se via fused DVE copy+accumulate (scratch dst reused)
        se = singles.tile([128, 2, BS], FP32)
        scratch = singles.tile([128, H, W], BF16)
        for s in range(BS):
            for t in range(2):
                nc.vector.tensor_scalar(
                    out=scratch,
                    in0=xall[:, s, t],
                    scalar1=1.0,
                    scalar2=None,
                    op0=mybir.AluOpType.mult,
                    op1=mybir.AluOpType.add,
                    accum_out=se[:, t, s:s + 1],
                )

        # ---- gates -> block-diagonal stationary G (M is (s, q) sample-major)
        se_rep = singles.tile([128, 2, 64], FP32)
        for t in range(2):
            for s in range(BS):
                nc.vector.tensor_scalar(
                    out=se_rep[:, t, NQ * s:NQ * (s + 1)], in0=ones16,
                    scalar1=se[:, t, s:s + 1], scalar2=None,
                    op0=mybir.AluOpType.mult)
        L = psS.tile([128, 64], FP32, tag="ps", name="Lpsum")
        for t in range(2):
            nc.tensor.matmul(L, lhsT=cst[:, 128 * t:128 * (t + 1)],
                             rhs=se_rep[:, t, :],
                             start=(t == 0), stop=(t == 1))
        g0 = singles.tile([128, 64], FP32)
        nc.scalar.activation(out=g0, in_=L,
                             func=mybir.ActivationFunctionType.Sigmoid,
                             bias=rbx, scale=1.0)
        G = singles.tile([128, 64], BF16)
        nc.vector.tensor_tensor(out=G, in0=g0, in1=mask, op=mybir.AluOpType.mult)

        # ---- per half: synthesize w_dyn on PE, SBUF re-gather, then conv
        for half in range(2):
            stage = stagep.tile([128, 8, 3 * NWIN], BF16, tag="stage",
                                name=f"stage{half}")
            for wloc in range(3):
                w = 3 * half + wloc
                for clp in range(8):
                    # MMs for cl=clp (u=0) and cl=clp+8 (u=1) share one
                    # (128,384) PSUM tile via partition halves; one drain
                    # covers both
                    ps = psS.tile([128, NWIN], FP32, tag="ps",
                                  name=f"ps{half}_{wloc}_{clp}")
                    nc.tensor.matmul(ps[0:64, :], lhsT=G,
                                     rhs=bkt[w][:, clp, :],
                                     start=True, stop=True)
                    nc.tensor.matmul(ps[64:128, :], lhsT=G,
                                     rhs=bkt[w][:, 8 + clp, :],
                                     start=True, stop=True)
                    dst = stage[:, clp, wloc * NWIN:(wloc + 1) * NWIN]
                    if clp % 2 == 0:
                        nc.scalar.activation(
                            out=dst, in_=ps,
                            func=mybir.ActivationFunctionType.Copy)
                    else:
                        nc.vector.tensor_copy(out=dst, in_=ps)

            wd = {}
            for s in range(BS):
                for t in range(2):
                    wdt = wdynp.tile([128, 9 * 128], BF16, tag=f"wd{half}{s}{t}")
                    # stage partition 64u+16s+8t+q' holds ci=(8t+q')*16+clp+8u
                    # for clp in the free dim; with the host-side ci
                    # permutation, partition d=64u+8q'+clp of the conv
                    # stationary IS that channel. Flat [64,1152]<-[8,8*1152]
                    # views keep each dst partition one contiguous 2.3KB run.
                    for u in range(2):
                        src = stage[64 * u + NQ * s + 8 * t:
                                    64 * u + NQ * s + 8 * t + 8]
                        eng = nc.sync if u == 0 else nc.scalar
                        eng.dma_start(out=wdt[64 * u:64 * (u + 1)], in_=src)
                    wd[s, t] = wdt

            # second-half bank DMAs issue only now, so half-0 re-gathers sat
            # ahead of them in the sync FIFO; 4.7MB then streams under conv h0
            if half == 0:
                for w in range(3, 6):
                    bk = bankp.tile([128, NQ, NWIN], BF16, tag="bk",
                                    name=f"bk{w}")
                    nc.sync.dma_start(out=bk, in_=bank_d[:, w])
                    bkt[w] = bk

            for s in range(BS):
                pst = [psC.tile([128, 14, W], FP32, tag="pc",
                                name=f"pc{half}_{s}_{c}") for c in range(2)]
                for t in range(2):
                    # k=4 (center, full view) first so every PSUM element is
                    # initialized by the start=True matmul; edge-clipped
                    # partial views then accumulate.
                    korder = (4, 0, 1, 2, 3, 5, 6, 7, 8) if t == 0 else range(9)
                    for k in korder:
                        kh, kw = divmod(k, 3)
                        lw = wd[s, t][:, k * 128:(k + 1) * 128]
                        xlo, xhi = max(0, 1 - kw), min(W - 1, W - kw)
                        for c in range(2):
                            ylo = max(c * 14, 1 - kh)
                            yhi = min(c * 14 + 13, H - kh)
                            rhs = xall[:, s, t,
                                       ylo + kh - 1:yhi + kh,
                                       xlo + kw - 1:xhi + kw]
                            nc.tensor.matmul(
                                pst[c][:, ylo - c * 14:yhi + 1 - c * 14,
                                       xlo:xhi + 1],
                                lhsT=lw, rhs=rhs,
                                start=(t == 0 and k == 4),
                                stop=(t == 1 and k == 8),
                            )
                ot = outp.tile([128, 2, 14, W], BF16, tag="ot",
                               name=f"ot{half}_{s}")
                for c in range(2):
                    nc.scalar.activation(
                        out=ot[:, c], in_=pst[c],
                        func=mybir.ActivationFunctionType.Identity,
                        bias=biasT[:, half:half + 1], scale=1.0)
                nc.scalar.dma_start(
                    out=out_d[s, half * 128:(half + 1) * 128], in_=ot)
    nc.finalize()
    return nc


# partition d (within a 128-channel tile) holds channel perm[d]:
# d = 64u + 8q' + clp  <->  ci_lo = 16q' + 8u + clp
CI_PERM = np.array([(d % 64) // 8 * 16 + (d // 64) * 8 + d % 8
                    for d in range(128)])
CI_MAP = np.concatenate([CI_PERM, 128 + CI_PERM])


def _host_prep(route_w, route_b, weight, bias):
    """Host-side layout transforms (pure numpy, replicated to every core)."""
    We = np.ascontiguousarray(weight.T).reshape(NUM, C_OUT, C_IN, K, K)
    Wf = We.transpose(0, 2, 1, 3, 4)            # [e, ci, co, kh, kw]
    Wf = Wf.reshape(NUM, C_IN, 2, 128, 9)       # [e, ci, co_t, co_lo, khkw]
    Wf = Wf.transpose(0, 1, 2, 4, 3)            # [e, ci, co_t, khkw, co_lo]
    Wf = Wf.reshape(NUM, C_IN, F)               # f = co_t*1152 + khkw*128 + co_lo
    Bk = Wf.reshape(NUM, NQ, NQ, 6, NWIN)       # [e, q, cl, w, n]
    bank = np.ascontiguousarray(
        Bk.transpose(1, 0, 3, 2, 4).reshape(128, 6, NQ, NWIN)).astype(BF16_NP)

    cst = np.empty((128, 323), np.float32)
    cst[:, 0:256] = np.tile((route_w / (H * W)).T, (1, NQ))[CI_MAP] \
        .reshape(2, 128, 128).transpose(1, 0, 2).reshape(128, 256)
    cst[:, 256] = np.tile(route_b, NQ)
    cst[:, 257:259] = bias.reshape(2, 128).T
    # G column m = (s, q): q(m) = m % 16
    cst[:, 259:323] = (np.arange(128)[:, None] // 8
                       == np.arange(64)[None, :] % NQ)
    return bank, cst


def _ensure_ntff_hook():
    """Provide antenv.axon_hooks (absent in this image) so trace=True works.

    The boot script ships a ctypes NTFF hook but can only register it through
    antenv.axon_hooks; shim that module and register the hook ourselves.
    """
    import sys
    import types
    try:
        from antenv.axon_hooks import get_axon_ntff_profile_hook  # noqa: F401
        return
    except ImportError:
        pass
    try:
        import antenv
        from trn_agent_boot.trn_boot import _ntff_profile_via_ctypes
    except ImportError:
        return
    mod = types.ModuleType("antenv.axon_hooks")
    holder = {"hook": None}
    mod.set_axon_ntff_profile_hook = lambda h: holder.__setitem__("hook", h)
    mod.get_axon_ntff_profile_hook = lambda: holder["hook"]
    sys.modules["antenv.axon_hooks"] = mod
    antenv.axon_hooks = mod
    mod.set_axon_ntff_profile_hook(
        _ntff_profile_via_ctypes("/opt/axon/libaxon_pjrt.so"))


_NC_CACHE = None


def kernel(inputs, route_w, route_b, weight, bias):
    global _NC_CACHE
    inputs = np.asarray(inputs, dtype=np.float32)
    route_w = np.asarray(route_w, dtype=np.float32)
    route_b = np.asarray(route_b, dtype=np.float32)
    weight = np.asarray(weight, dtype=np.float32)
    bias = np.asarray(bias, dtype=np.float32)

    bank, cst = _host_prep(route_w, route_b, weight, bias)

    if _NC_CACHE is None:
        _NC_CACHE = build_nc()
    nc = _NC_CACHE

    shared = {"bank": bank, "cst": cst}
    # [B, 256, H, W] -> per-core [128, BS, 2, H, W], channel = t*128+perm(p)
    x16 = inputs[:, CI_MAP].astype(BF16_NP)
    x16 = x16.reshape(B, 2, 128, H, W)
    in_maps = []
    for c in range(N_CORES):
        xc = x16[BS * c:BS * (c + 1)]            # [BS, 2, 128, H, W]
        xc = np.ascontiguousarray(xc.transpose(2, 0, 1, 3, 4))
        in_maps.append({"x": xc, **shared})
    trace = bool(int(os.environ.get("KERNEL_TRACE", "0")))
    if trace:
        _ensure_ntff_hook()
    res = run_bass_kernel_spmd(
        nc, in_maps, core_ids=list(range(N_CORES)), trace=trace,
        tmpdir=os.environ.get("KERNEL_TMPDIR"),
    )
    out = np.concatenate([res.results[c]["out"] for c in range(N_CORES)],
                         axis=0).astype(np.float32)
    kernel.last_results = res
    return out


kernel.last_results = None

